# revision 32
# baseline (speedup 1.0000x reference)
"""Trainium2 Bass kernel for nn_Encoder_68977174774136 (heterogeneous GCN encoder).

Math (per batch b):
    X = node_features @ W_embed                       # [N, H]
    adj_n[c] = adj[c] / max(rowsum(adj[c]), 1)        # [N, N] per edge type
    4 layers (2 stacks x 2):
        xw[c] = X @ (W[l,c]/6)                        # 1/6 = (1-alpha)/C
        X = relu(X @ (0.5*Ws[l]) + sum_c adj_n[c] @ xw[c])
    h = [mean, max, std(ddof=1)] over nodes           # [3H]
    return h, X[:, :-2], adj

Sharding: data-parallel over batch, 16 graphs per NeuronCore x 8 cores.

Device strategy (per core):
  - Everything is feature-major on chip: X_t = X^T is [H=128, N] so H sits
    exactly on the 128 partitions; matmuls contract over partitions.
  - The aggregation adj_n @ xw needs adjacency with the contraction index j on
    partitions, i.e. transposed.  The host stages adj as adj^T (a pure layout
    transform, like the node-feature transpose); ALL math on it (degree,
    normalization, message passing) runs on device:
      deg[i]  = ones^T-matmul over the j-partitioned tiles (fp32r, exact)
      inv     = 1/max(deg, 1)                          (DVE)
      inv_rep = ones x inv rank-1 matmul               (PE broadcast)
      adj_nt  = adjT * inv_rep -> bf16                 (DVE, written in place
                into SBUF-resident tiles)
    Normalized transposed adjacency lives in SBUF (bf16) in an 8-batch
    rotating window, so the 4 GCN layers never re-touch HBM for adjacency.
  - All heavy matmuls are bf16 (PE runs plain fp32 at 1/4 rate), accumulated
    in f32 PSUM.  alpha/C constants are folded into the weights host-side.
  - Batches flow through a software-pipelined group wavefront (groups of 4,
    layer-major inside a group) with preprocessing for group g+1 interleaved
    between group g's layer units, so every engine's in-order stream mixes
    the two pipeline stages.
"""

import sys

sys.path.insert(0, "/opt/trn_rl_repo")

from contextlib import ExitStack

import ml_dtypes
import numpy as np

import concourse.bacc as bacc
import concourse.mybir as mybir
import concourse.tile as tile
from concourse import masks
from concourse.bass_utils import run_bass_kernel_spmd

BB = 16  # batches per core
NCORES = 8
N = 402  # nodes
C = 3  # edge categories
H = 128  # hidden size == SBUF partitions
FIN = 6  # raw feature dim
NL = 4  # total GCN layers (2 stacks x 2)

F32 = mybir.dt.float32
FP8 = mybir.dt.float8e4
F16 = mybir.dt.float16
BF16 = mybir.dt.bfloat16
AF = mybir.ActivationFunctionType
AX = mybir.AxisListType
OP = mybir.AluOpType

# j (and i) tiled into chunks of <=128 for the 128-wide contraction
JCHUNKS = [(0, 128), (128, 128), (256, 128), (384, N - 384)]


def build_nc(rounds=1):
    nc = bacc.Bacc("TRN2", target_bir_lowering=False, debug=False)

    adjt = nc.dram_tensor("adjt", [BB, C, N, N], FP8, kind="ExternalInput")
    nf = nc.dram_tensor("nf", [BB, FIN, N], BF16, kind="ExternalInput")
    wmsg = nc.dram_tensor("wmsg", [NL, H, C * H], BF16, kind="ExternalInput")
    wself = nc.dram_tensor("wself", [NL, H, H], BF16, kind="ExternalInput")
    wemb = nc.dram_tensor("wemb", [FIN, H], BF16, kind="ExternalInput")
    enc_out = nc.dram_tensor("enc_out", [BB, N, H], F32, kind="ExternalOutput")
    h_out = nc.dram_tensor("h_out", [BB, 3 * H], F32, kind="ExternalOutput")

    with tile.TileContext(nc) as tc, ExitStack() as ctx:
        persist = ctx.enter_context(tc.tile_pool(name="persist", bufs=1))
        work = ctx.enter_context(tc.tile_pool(name="work", bufs=2))
        psp = ctx.enter_context(tc.tile_pool(name="psp", bufs=1, space="PSUM"))

        # ---- persistent tiles -----------------------------------------------
        X_t = [persist.tile([H, N], BF16, name=f"xt_{b}", tag=f"xt_{b}") for b in range(BB)]
        wmsg_sb = [
            persist.tile([H, C * H], BF16, name=f"wmsg_{l}", tag=f"wmsg_{l}") for l in range(NL)
        ]
        wself_sb = [
            persist.tile([H, H], BF16, name=f"wself_{l}", tag=f"wself_{l}") for l in range(NL)
        ]
        wemb_sb = persist.tile([FIN, H], BF16, name="wemb_sb", tag="wemb_sb")
        ident = persist.tile([H, H], F32, name="ident", tag="ident")
        ones_col = persist.tile([H, 1], FP8, name="ones_col", tag="ones_col")
        ones_row = persist.tile([1, H], F16, name="ones_row", tag="ones_row")
        s_sum = persist.tile([H, BB], F32, name="s_sum", tag="s_sum")
        s_max = persist.tile([H, BB], F32, name="s_max", tag="s_max")
        s_sq = persist.tile([H, BB], F32, name="s_sq", tag="s_sq")
        stats_cat = persist.tile([H, 3 * BB], F32, name="stats_cat", tag="stats_cat")
        tmp_a = persist.tile([H, BB], F32, name="tmp_a", tag="tmp_a")
        tmp_b = persist.tile([H, BB], F32, name="tmp_b", tag="tmp_b")

        # adj_nt[(b, c)][p, jc, i] = adj_n[b, c, i, jc*128+p] (bf16).  Only ~2
        # batch-groups are live at once, so slots rotate on b mod 8; Tile's
        # WAR tracking delays the normalize write of b+8 until the msg
        # matmuls of b finished reading the slot.
        adj_nt = {}

        # ---- constants and weights ------------------------------------------
        masks.make_identity(nc, ident[:, :])
        nc.gpsimd.memset(ones_col[:, :], 1.0)
        nc.gpsimd.memset(ones_row[:, :], 1.0)
        nc.sync.dma_start(wemb_sb[:, :], wemb[:, :])
        for l in range(NL):
            nc.sync.dma_start(wmsg_sb[l][:, :], wmsg[l])
            nc.sync.dma_start(wself_sb[l][:, :], wself[l])

        def act_copy(out, in_):
            nc.scalar.activation(out, in_, AF.Copy)

        # ---- per-(b, c) preprocessing unit ----------------------------------
        pre_count = [0]

        def pre_unit_a(b, c):
            if c == 0:
                # embed: X_t[b] = (nf[b] @ W_embed)^T via f-contraction
                nfs = work.tile([FIN, N], BF16, tag="nfsb", bufs=2)
                nc.sync.dma_start(nfs[:, :], nf[b])
                emb_ps = psp.tile([H, N], F32, tag="mm", bufs=2)
                nc.tensor.matmul(
                    emb_ps[:, :], wemb_sb[:, :], nfs[:, :], start=True, stop=True
                )
                act_copy(X_t[b][:, :], emb_ps[:, :])

            # adjT[b, c] rows are j; chunks land on partitions
            stage = work.tile([H, 4, N], FP8, tag="stage", bufs=8)
            if pre_count[0] < 8:
                # first touch of each pool slot: the chunk-3 tail partitions
                # are never DMA'd; init once so the deg matmul AP is clean
                nc.gpsimd.memset(stage[:, 3, :], 0.0)
            pre_count[0] += 1
            nc.sync.dma_start(
                stage[:, 0:3, :],
                adjt[b, c, 0:384, :].rearrange("(jc p) i -> p jc i", p=128),
            )
            nc.sync.dma_start(stage[0 : N - 384, 3, :], adjt[b, c, 384:N, :])

            # deg[i] = sum_j adjT[j, i] via ones-matmul (fp8 operands are
            # exact 0/1; f32 PSUM accumulation makes the counts exact)
            deg_ps = psp.tile([1, N], F32, tag="bc", bufs=3)
            for jc, (j0, jsz) in enumerate(JCHUNKS):
                nc.tensor.matmul(
                    deg_ps[0:1, :],
                    ones_col[0:jsz, :],
                    stage[0:jsz, jc, :],
                    start=(jc == 0),
                    stop=(jc == 3),
                )
            inv = work.tile([1, N], F32, tag="inv", bufs=4)
            nc.vector.tensor_scalar_max(inv[0:1, :], deg_ps[0:1, :], 1.0)
            nc.vector.reciprocal(inv[0:1, :], inv[0:1, :])
            inv16 = work.tile([1, N], F16, tag="inv16", bufs=4)
            nc.vector.tensor_copy(inv16[0:1, :], inv[0:1, :])
            return stage, inv16

        def pre_unit_b(b, c, stage, inv16):
            # replicate inv across partitions with a rank-1 f16 matmul (a full
            # wavefront step after the recip, so PE never waits on DVE here)
            bc_ps = psp.tile([H, N], F32, tag="bc", bufs=3)
            nc.tensor.matmul(
                bc_ps[:, :],
                ones_row[0:1, :],
                inv16[0:1, :],
                start=True,
                stop=True,
            )
            # normalize + cast straight into the SBUF-resident transposed tile
            ant = work.tile(
                [H, 4, N], BF16, name=f"adjnt_{b}_{c}", tag=f"adjnt_{b % 8}_{c}", bufs=1
            )
            adj_nt[(b, c)] = ant
            for jc, (j0, jsz) in enumerate(JCHUNKS):
                nc.vector.tensor_tensor(
                    ant[0:jsz, jc, 0:N],
                    stage[0:jsz, jc, :],
                    bc_ps[0:jsz, :],
                    op=OP.mult,
                )

        # ---- per-(b, l) layer unit ------------------------------------------
        def layer_unit(b, l):
            xws = []
            for jc, (j0, jsz) in enumerate(JCHUNKS):
                xw_ps = psp.tile([H, C * H], F32, tag="xw", bufs=3)
                nc.tensor.matmul(
                    xw_ps[0:jsz, :],
                    X_t[b][:, j0 : j0 + jsz],
                    wmsg_sb[l][:, :],
                    start=True,
                    stop=True,
                )
                xw_sb = work.tile([H, C * H], BF16, tag="xwsb", bufs=8)
                act_copy(xw_sb[0:jsz, :], xw_ps[0:jsz, :])
                xws.append(xw_sb)

            mm_ps = psp.tile([H, N], F32, tag="mm", bufs=2)
            nc.tensor.matmul(
                mm_ps[:, :], wself_sb[l][:, :], X_t[b][:, :], start=True, stop=False
            )
            cnt = 0
            for c in range(C):
                for jc, (j0, jsz) in enumerate(JCHUNKS):
                    cnt += 1
                    nc.tensor.matmul(
                        mm_ps[:, :],
                        xws[jc][0:jsz, c * H : (c + 1) * H],
                        adj_nt[(b, c)][0:jsz, jc, 0:N],
                        start=False,
                        stop=(cnt == C * 4),
                    )

            if l < NL - 1:
                nc.scalar.activation(X_t[b][:, :], mm_ps[:, :], AF.Relu)
            else:
                # final layer: f32 activations + readout stats + output
                xf = work.tile([H, N], F32, tag="xf32", bufs=2)
                nc.scalar.activation(
                    xf[:, :], mm_ps[:, :], AF.Relu, accum_out=s_sum[:, b : b + 1]
                )
                nc.vector.reduce_max(s_max[:, b : b + 1], xf[:, :], axis=AX.X)
                junk = work.tile([H, N], BF16, tag="junk", bufs=2)
                nc.scalar.activation(
                    junk[:, 0:N], xf[:, :], AF.Square, accum_out=s_sq[:, b : b + 1]
                )
                est = work.tile([H, 4, H], F32, tag="encst", bufs=2)
                for ic, (i0, isz) in enumerate(JCHUNKS):
                    tp_ps = psp.tile([H, H], F32, tag="mm", bufs=2)
                    nc.tensor.transpose(tp_ps[0:isz, :], xf[:, i0 : i0 + isz], ident[:, :])
                    nc.vector.tensor_copy(est[0:isz, ic, :], tp_ps[0:isz, :])
                nc.sync.dma_start(
                    enc_out[b, 0:384, :].rearrange("(ic p) g -> p ic g", p=128),
                    est[:, 0:3, :],
                )
                nc.sync.dma_start(enc_out[b, 384:N, :], est[0 : N - 384, 3, :])

        # ---- software pipeline: group wavefront -----------------------------
        # Preprocessing is split: _a (loads + degree) and _b (broadcast +
        # normalize) are emitted one wavefront step apart so PE's in-order
        # stream never waits on the DVE reciprocal.
        GW = 4
        NSTEP = NL * GW
        for rnd in range(rounds):
            pre_list = [(b, c) for b in range(BB) for c in range(C)]
            pi = 0
            pending = {}
            while pi < GW * C:  # prologue: group 0
                st = pre_unit_a(*pre_list[pi])
                pre_unit_b(*pre_list[pi], *st)
                pi += 1
            for g in range(BB // GW):
                # schedule group g+1's 12 pre-units across this group's 16
                # layer steps: _a at step u*16//12, _b one step later
                sched = [[] for _ in range(NSTEP)]
                if pi < len(pre_list):
                    for u in range(GW * C):
                        sa = u * NSTEP // (GW * C)
                        sched[sa].append(("a", pre_list[pi + u]))
                        sched[min(sa + 1, NSTEP - 1)].append(("b", pre_list[pi + u]))
                    pi += GW * C
                step = 0
                for l in range(NL):
                    for b in range(g * GW, (g + 1) * GW):
                        layer_unit(b, l)
                        for kind, unit in sched[step]:
                            if kind == "a":
                                pending[unit] = pre_unit_a(*unit)
                            else:
                                pre_unit_b(*unit, *pending.pop(unit))
                        step += 1

        # ---- readout h = [mean | max | std] ---------------------------------
        nc.vector.tensor_scalar_mul(stats_cat[:, 0:BB], s_sum[:, :], 1.0 / N)
        nc.vector.tensor_copy(stats_cat[:, BB : 2 * BB], s_max[:, :])
        # var = ss/(N-1) - s^2/(N*(N-1)), clamped at 0
        nc.vector.tensor_mul(tmp_a[:, :], s_sum[:, :], s_sum[:, :])
        nc.vector.tensor_scalar_mul(tmp_b[:, :], s_sq[:, :], 1.0 / (N - 1))
        nc.vector.scalar_tensor_tensor(
            tmp_a[:, :], tmp_a[:, :], -1.0 / (N * (N - 1)), tmp_b[:, :], OP.mult, OP.add
        )
        nc.vector.tensor_scalar_max(tmp_a[:, :], tmp_a[:, :], 0.0)
        nc.scalar.activation(stats_cat[:, 2 * BB : 3 * BB], tmp_a[:, :], AF.Sqrt)

        tp_ps = psp.tile([H, H], F32, tag="mm", bufs=2)
        nc.tensor.transpose(tp_ps[0 : 3 * BB, :], stats_cat[:, :], ident[:, :])
        statsT = work.tile([H, H], F32, tag="statsT", bufs=1)
        nc.vector.tensor_copy(statsT[0 : 3 * BB, :], tp_ps[0 : 3 * BB, :])
        for s in range(3):
            nc.sync.dma_start(
                h_out[0:BB, s * H : (s + 1) * H], statsT[s * BB : (s + 1) * BB, 0:H]
            )

    nc.compile()
    return nc


_NC_CACHE = None


def get_nc():
    global _NC_CACHE
    if _NC_CACHE is None:
        _NC_CACHE = build_nc()
    return _NC_CACHE


def make_in_maps(node_features, adj, W_embed, W1, Ws1, W2, Ws2):
    node_features = np.asarray(node_features, dtype=np.float32)
    adj = np.asarray(adj, dtype=np.float32)
    W_embed = np.asarray(W_embed, dtype=np.float32)
    W1 = np.asarray(W1, dtype=np.float32)
    Ws1 = np.asarray(Ws1, dtype=np.float32)
    W2 = np.asarray(W2, dtype=np.float32)
    Ws2 = np.asarray(Ws2, dtype=np.float32)

    # layout staging only (no math): adjacency transposed so the contraction
    # index j is outermost, features transposed to feature-major
    adjT = np.ascontiguousarray(np.transpose(adj, (0, 1, 3, 2))).astype(
        ml_dtypes.float8_e4m3fn
    )
    nfT = np.ascontiguousarray(np.transpose(node_features, (0, 2, 1))).astype(
        ml_dtypes.bfloat16
    )
    # [NL, H, C*H]: (1-alpha)/C folded in; free dim is (c, g) c-major
    wm = np.concatenate([W1, W2], axis=0)  # [NL, C, H, H]
    wm = np.ascontiguousarray(
        (np.transpose(wm, (0, 2, 1, 3)) / 6.0).reshape(NL, H, C * H)
    ).astype(ml_dtypes.bfloat16)
    ws = (np.concatenate([Ws1, Ws2], axis=0) * 0.5).astype(ml_dtypes.bfloat16)
    we = W_embed.astype(ml_dtypes.bfloat16)

    in_maps = []
    for cid in range(NCORES):
        sl = slice(cid * BB, (cid + 1) * BB)
        in_maps.append(
            {
                "adjt": np.ascontiguousarray(adjT[sl]),
                "nf": np.ascontiguousarray(nfT[sl]),
                "wmsg": wm,
                "wself": ws,
                "wemb": we,
            }
        )
    return in_maps


def run(node_features, adj, W_embed, W1, Ws1, W2, Ws2, **spmd_kwargs):
    nc = get_nc()
    in_maps = make_in_maps(node_features, adj, W_embed, W1, Ws1, W2, Ws2)
    res = run_bass_kernel_spmd(nc, in_maps, core_ids=list(range(NCORES)), **spmd_kwargs)
    h = np.concatenate([r["h_out"] for r in res.results], axis=0)
    enc = np.concatenate([r["enc_out"] for r in res.results], axis=0)[:, : N - 2, :]
    return (h, enc, np.asarray(adj, dtype=np.float32)), res


def kernel(node_features, adj, W_embed, W1, Ws1, W2, Ws2):
    out, _ = run(node_features, adj, W_embed, W1, Ws1, W2, Ws2)
    return out


# revision 37
# speedup vs baseline: 1.2789x; 1.2789x over previous
"""Trainium2 Bass kernel for nn_Encoder_68977174774136 (heterogeneous GCN encoder).

Math (per batch b):
    X = node_features @ W_embed                       # [N, H]
    adj_n[c] = adj[c] / max(rowsum(adj[c]), 1)        # [N, N] per edge type
    4 layers (2 stacks x 2):
        xw[c] = X @ (W[l,c]/6)                        # 1/6 = (1-alpha)/C
        X = relu(X @ (0.5*Ws[l]) + sum_c adj_n[c] @ xw[c])
    h = [mean, max, std(ddof=1)] over nodes           # [3H]
    return h, X[:, :-2], adj

Sharding: data-parallel over batch, 16 graphs per NeuronCore x 8 cores.

Device strategy (per core):
  - Everything is feature-major on chip: X_t = X^T is [H=128, N] so H sits
    exactly on the 128 partitions; matmuls contract over partitions.
  - The aggregation adj_n @ xw needs adjacency with the contraction index j on
    partitions, i.e. transposed.  The host stages adj as adj^T (a pure layout
    transform, like the node-feature transpose); ALL math on it (degree,
    normalization, message passing) runs on device:
      deg[i]  = ones^T-matmul over the j-partitioned tiles (fp32r, exact)
      inv     = 1/max(deg, 1)                          (DVE)
      inv_rep = ones x inv rank-1 matmul               (PE broadcast)
      adj_nt  = adjT * inv_rep -> bf16                 (DVE, written in place
                into SBUF-resident tiles)
    Normalized transposed adjacency lives in SBUF (bf16) in an 8-batch
    rotating window, so the 4 GCN layers never re-touch HBM for adjacency.
  - All heavy matmuls are bf16 (PE runs plain fp32 at 1/4 rate), accumulated
    in f32 PSUM.  alpha/C constants are folded into the weights host-side.
  - Batches flow through a software-pipelined group wavefront (groups of 4,
    layer-major inside a group) with preprocessing for group g+1 interleaved
    between group g's layer units, so every engine's in-order stream mixes
    the two pipeline stages.
"""

import sys

sys.path.insert(0, "/opt/trn_rl_repo")

from contextlib import ExitStack

import ml_dtypes
import numpy as np

import concourse.bacc as bacc
import concourse.mybir as mybir
import concourse.tile as tile
from concourse import masks
from concourse.bass_utils import run_bass_kernel_spmd

BB = 16  # batches per core
NCORES = 8
N = 402  # nodes
C = 3  # edge categories
H = 128  # hidden size == SBUF partitions
FIN = 6  # raw feature dim
NL = 4  # total GCN layers (2 stacks x 2)

F32 = mybir.dt.float32
FP8 = mybir.dt.float8e4
F16 = mybir.dt.float16
BF16 = mybir.dt.bfloat16
AF = mybir.ActivationFunctionType
AX = mybir.AxisListType
OP = mybir.AluOpType

# j (and i) tiled into chunks of <=128 for the 128-wide contraction
JCHUNKS = [(0, 128), (128, 128), (256, 128), (384, N - 384)]


def build_nc(rounds=1):
    nc = bacc.Bacc("TRN2", target_bir_lowering=False, debug=False)

    adjt = nc.dram_tensor("adjt", [BB, C, N, N], FP8, kind="ExternalInput")
    nf = nc.dram_tensor("nf", [BB, FIN, N], BF16, kind="ExternalInput")
    wmsg = nc.dram_tensor("wmsg", [NL, H, C * H], BF16, kind="ExternalInput")
    wself = nc.dram_tensor("wself", [NL, H, H], BF16, kind="ExternalInput")
    wemb = nc.dram_tensor("wemb", [FIN, H], BF16, kind="ExternalInput")
    enc_out = nc.dram_tensor("enc_out", [BB, N, H], F32, kind="ExternalOutput")
    h_out = nc.dram_tensor("h_out", [BB, 3 * H], F32, kind="ExternalOutput")

    with tile.TileContext(nc) as tc, ExitStack() as ctx:
        persist = ctx.enter_context(tc.tile_pool(name="persist", bufs=1))
        work = ctx.enter_context(tc.tile_pool(name="work", bufs=2))
        psp = ctx.enter_context(tc.tile_pool(name="psp", bufs=1, space="PSUM"))

        # ---- persistent tiles -----------------------------------------------
        X_t = [persist.tile([H, N], BF16, name=f"xt_{b}", tag=f"xt_{b}") for b in range(BB)]
        wmsg_sb = [
            persist.tile([H, C * H], BF16, name=f"wmsg_{l}", tag=f"wmsg_{l}") for l in range(NL)
        ]
        wself_sb = [
            persist.tile([H, H], BF16, name=f"wself_{l}", tag=f"wself_{l}") for l in range(NL)
        ]
        wemb_sb = persist.tile([FIN, H], BF16, name="wemb_sb", tag="wemb_sb")
        ident = persist.tile([H, H], F32, name="ident", tag="ident")
        ones_col = persist.tile([H, 1], FP8, name="ones_col", tag="ones_col")
        ones_row = persist.tile([1, H], F16, name="ones_row", tag="ones_row")
        s_sum = persist.tile([H, BB], F32, name="s_sum", tag="s_sum")
        s_max = persist.tile([H, BB], F32, name="s_max", tag="s_max")
        s_sq = persist.tile([H, BB], F32, name="s_sq", tag="s_sq")
        stats_cat = persist.tile([H, 3 * BB], F32, name="stats_cat", tag="stats_cat")
        tmp_a = persist.tile([H, BB], F32, name="tmp_a", tag="tmp_a")
        tmp_b = persist.tile([H, BB], F32, name="tmp_b", tag="tmp_b")

        # adj_nt[(b, c)][p, jc, i] = adj_n[b, c, i, jc*128+p] (bf16).  Only ~2
        # batch-groups are live at once, so slots rotate on b mod 8; Tile's
        # WAR tracking delays the normalize write of b+8 until the msg
        # matmuls of b finished reading the slot.
        adj_nt = {}

        # ---- constants and weights ------------------------------------------
        masks.make_identity(nc, ident[:, :])
        nc.gpsimd.memset(ones_col[:, :], 1.0)
        nc.gpsimd.memset(ones_row[:, :], 1.0)
        nc.sync.dma_start(wemb_sb[:, :], wemb[:, :])
        for l in range(NL):
            nc.sync.dma_start(wmsg_sb[l][:, :], wmsg[l])
            nc.sync.dma_start(wself_sb[l][:, :], wself[l])

        def act_copy(out, in_):
            nc.scalar.activation(out, in_, AF.Copy)

        # ---- per-(b, c) preprocessing unit ----------------------------------
        pre_count = [0]

        def embed_unit(b):
            # embed: X_t[b] = (nf[b] @ W_embed)^T via f-contraction
            nfs = work.tile([FIN, N], BF16, tag="nfsb", bufs=2)
            nc.sync.dma_start(nfs[:, :], nf[b])
            emb_ps = psp.tile([H, N], F32, tag="mm", bufs=3)
            nc.tensor.matmul(
                emb_ps[:, :], wemb_sb[:, :], nfs[:, :], start=True, stop=True
            )
            act_copy(X_t[b][:, :], emb_ps[:, :])

        def pre_unit_a(b, c):
            # adjT[b, c] rows are j; chunks land on partitions
            stage = work.tile([H, 4, N], FP8, tag="stage", bufs=8)
            if pre_count[0] < 8:
                # first touch of each pool slot: the chunk-3 tail partitions
                # are never DMA'd; init once so the deg matmul AP is clean
                nc.gpsimd.memset(stage[:, 3, :], 0.0)
            pre_count[0] += 1
            nc.sync.dma_start(
                stage[:, 0:3, :],
                adjt[b, c, 0:384, :].rearrange("(jc p) i -> p jc i", p=128),
            )
            nc.sync.dma_start(stage[0 : N - 384, 3, :], adjt[b, c, 384:N, :])

            # deg[i] = sum_j adjT[j, i] via ones-matmul (fp8 operands are
            # exact 0/1; f32 PSUM accumulation makes the counts exact)
            deg_ps = psp.tile([1, N], F32, tag="bc", bufs=2)
            for jc, (j0, jsz) in enumerate(JCHUNKS):
                nc.tensor.matmul(
                    deg_ps[0:1, :],
                    ones_col[0:jsz, :],
                    stage[0:jsz, jc, :],
                    start=(jc == 0),
                    stop=(jc == 3),
                )
            inv = work.tile([1, N], F32, tag="inv", bufs=4)
            nc.vector.tensor_scalar_max(inv[0:1, :], deg_ps[0:1, :], 1.0)
            nc.vector.reciprocal(inv[0:1, :], inv[0:1, :])
            inv16 = work.tile([1, N], F16, tag="inv16", bufs=4)
            nc.vector.tensor_copy(inv16[0:1, :], inv[0:1, :])
            return stage, inv16

        def pre_unit_b(b, c, stage, inv16):
            # replicate inv across partitions with a rank-1 f16 matmul (a full
            # wavefront step after the recip, so PE never waits on DVE here)
            bc_ps = psp.tile([H, N], F32, tag="bc", bufs=2)
            nc.tensor.matmul(
                bc_ps[:, :],
                ones_row[0:1, :],
                inv16[0:1, :],
                start=True,
                stop=True,
            )
            # normalize + cast straight into the SBUF-resident transposed tile
            ant = work.tile(
                [H, 4, N], BF16, name=f"adjnt_{b}_{c}", tag=f"adjnt_{b % 8}_{c}", bufs=1
            )
            adj_nt[(b, c)] = ant
            for jc, (j0, jsz) in enumerate(JCHUNKS):
                nc.vector.tensor_tensor(
                    ant[0:jsz, jc, 0:N],
                    stage[0:jsz, jc, :],
                    bc_ps[0:jsz, :],
                    op=OP.mult,
                )

        # ---- per-(b, l) layer unit ------------------------------------------
        def layer_unit(b, l):
            xws = []
            for jc, (j0, jsz) in enumerate(JCHUNKS):
                xw_ps = psp.tile([H, C * H], F32, tag="xw", bufs=3)
                nc.tensor.matmul(
                    xw_ps[0:jsz, :],
                    X_t[b][:, j0 : j0 + jsz],
                    wmsg_sb[l][:, :],
                    start=True,
                    stop=True,
                )
                xw_sb = work.tile([H, C * H], BF16, tag="xwsb", bufs=8)
                act_copy(xw_sb[0:jsz, :], xw_ps[0:jsz, :])
                xws.append(xw_sb)

            mm_ps = psp.tile([H, N], F32, tag="mm", bufs=3)
            nc.tensor.matmul(
                mm_ps[:, :], wself_sb[l][:, :], X_t[b][:, :], start=True, stop=False
            )
            cnt = 0
            # jc-outer so the first message matmuls only need xw chunk 0's
            # PSUM->SBUF copy; chunk k's copy has k*3*402 cycles of cover
            for jc, (j0, jsz) in enumerate(JCHUNKS):
                for c in range(C):
                    cnt += 1
                    nc.tensor.matmul(
                        mm_ps[:, :],
                        xws[jc][0:jsz, c * H : (c + 1) * H],
                        adj_nt[(b, c)][0:jsz, jc, 0:N],
                        start=False,
                        stop=(cnt == C * 4),
                    )

            if l < NL - 1:
                nc.scalar.activation(X_t[b][:, :], mm_ps[:, :], AF.Relu)
                return
            # final layer: f32 activations + readout stats + output
            xf = work.tile([H, N], F32, tag="xf32", bufs=2)
            nc.scalar.activation(
                xf[:, :], mm_ps[:, :], AF.Relu, accum_out=s_sum[:, b : b + 1]
            )
            nc.vector.reduce_max(s_max[:, b : b + 1], xf[:, :], axis=AX.X)
            junk = work.tile([H, N], BF16, tag="junk", bufs=2)
            nc.scalar.activation(
                junk[:, 0:N], xf[:, :], AF.Square, accum_out=s_sq[:, b : b + 1]
            )
            est = work.tile([H, 4, H], F32, tag="encst", bufs=2)
            for ic, (i0, isz) in enumerate(JCHUNKS):
                tp_ps = psp.tile([H, H], F32, tag="mm", bufs=3)
                nc.tensor.transpose(tp_ps[0:isz, :], xf[:, i0 : i0 + isz], ident[:, :])
                nc.vector.tensor_copy(est[0:isz, ic, :], tp_ps[0:isz, :])
            nc.sync.dma_start(
                enc_out[b, 0:384, :].rearrange("(ic p) g -> p ic g", p=128),
                est[:, 0:3, :],
            )
            nc.sync.dma_start(enc_out[b, 384:N, :], est[0 : N - 384, 3, :])

        # ---- software pipeline: group wavefront -----------------------------
        # Preprocessing is split: _a (loads + degree) and _b (broadcast +
        # normalize) are emitted one wavefront step apart so PE's in-order
        # stream never waits on the DVE reciprocal.
        GW = 4
        NSTEP = NL * GW
        for rnd in range(rounds):
            for b in range(BB):
                embed_unit(b)
            pre_list = [(b, c) for b in range(BB) for c in range(C)]
            pi = 0
            pending = {}
            while pi < GW * C:  # prologue: group 0
                st = pre_unit_a(*pre_list[pi])
                pre_unit_b(*pre_list[pi], *st)
                pi += 1
            for g in range(BB // GW):
                # schedule group g+1's 12 pre-units across this group's 16
                # layer steps: _a at step u*16//12, _b one step later
                sched = [[] for _ in range(NSTEP)]
                if pi < len(pre_list):
                    for u in range(GW * C):
                        sa = u * NSTEP // (GW * C)
                        sched[sa].append(("a", pre_list[pi + u]))
                        sched[min(sa + 1, NSTEP - 1)].append(("b", pre_list[pi + u]))
                    pi += GW * C
                step = 0
                for l in range(NL):
                    for b in range(g * GW, (g + 1) * GW):
                        layer_unit(b, l)
                        for kind, unit in sched[step]:
                            if kind == "a":
                                pending[unit] = pre_unit_a(*unit)
                            else:
                                pre_unit_b(*unit, *pending.pop(unit))
                        step += 1

        # ---- readout h = [mean | max | std] ---------------------------------
        nc.vector.tensor_scalar_mul(stats_cat[:, 0:BB], s_sum[:, :], 1.0 / N)
        nc.vector.tensor_copy(stats_cat[:, BB : 2 * BB], s_max[:, :])
        # var = ss/(N-1) - s^2/(N*(N-1)), clamped at 0
        nc.vector.tensor_mul(tmp_a[:, :], s_sum[:, :], s_sum[:, :])
        nc.vector.tensor_scalar_mul(tmp_b[:, :], s_sq[:, :], 1.0 / (N - 1))
        nc.vector.scalar_tensor_tensor(
            tmp_a[:, :], tmp_a[:, :], -1.0 / (N * (N - 1)), tmp_b[:, :], OP.mult, OP.add
        )
        nc.vector.tensor_scalar_max(tmp_a[:, :], tmp_a[:, :], 0.0)
        nc.scalar.activation(stats_cat[:, 2 * BB : 3 * BB], tmp_a[:, :], AF.Sqrt)

        tp_ps = psp.tile([H, H], F32, tag="mm", bufs=3)
        nc.tensor.transpose(tp_ps[0 : 3 * BB, :], stats_cat[:, :], ident[:, :])
        statsT = work.tile([H, H], F32, tag="statsT", bufs=1)
        nc.vector.tensor_copy(statsT[0 : 3 * BB, :], tp_ps[0 : 3 * BB, :])
        for s in range(3):
            nc.sync.dma_start(
                h_out[0:BB, s * H : (s + 1) * H], statsT[s * BB : (s + 1) * BB, 0:H]
            )

    nc.compile()
    return nc


_NC_CACHE = None


def get_nc():
    global _NC_CACHE
    if _NC_CACHE is None:
        _NC_CACHE = build_nc()
    return _NC_CACHE


def make_in_maps(node_features, adj, W_embed, W1, Ws1, W2, Ws2):
    node_features = np.asarray(node_features, dtype=np.float32)
    adj = np.asarray(adj, dtype=np.float32)
    W_embed = np.asarray(W_embed, dtype=np.float32)
    W1 = np.asarray(W1, dtype=np.float32)
    Ws1 = np.asarray(Ws1, dtype=np.float32)
    W2 = np.asarray(W2, dtype=np.float32)
    Ws2 = np.asarray(Ws2, dtype=np.float32)

    # layout staging only (no math): adjacency transposed so the contraction
    # index j is outermost, features transposed to feature-major
    adjT = np.ascontiguousarray(np.transpose(adj, (0, 1, 3, 2))).astype(
        ml_dtypes.float8_e4m3fn
    )
    nfT = np.ascontiguousarray(np.transpose(node_features, (0, 2, 1))).astype(
        ml_dtypes.bfloat16
    )
    # [NL, H, C*H]: (1-alpha)/C folded in; free dim is (c, g) c-major
    wm = np.concatenate([W1, W2], axis=0)  # [NL, C, H, H]
    wm = np.ascontiguousarray(
        (np.transpose(wm, (0, 2, 1, 3)) / 6.0).reshape(NL, H, C * H)
    ).astype(ml_dtypes.bfloat16)
    ws = (np.concatenate([Ws1, Ws2], axis=0) * 0.5).astype(ml_dtypes.bfloat16)
    we = W_embed.astype(ml_dtypes.bfloat16)

    in_maps = []
    for cid in range(NCORES):
        sl = slice(cid * BB, (cid + 1) * BB)
        in_maps.append(
            {
                "adjt": np.ascontiguousarray(adjT[sl]),
                "nf": np.ascontiguousarray(nfT[sl]),
                "wmsg": wm,
                "wself": ws,
                "wemb": we,
            }
        )
    return in_maps


def run(node_features, adj, W_embed, W1, Ws1, W2, Ws2, **spmd_kwargs):
    nc = get_nc()
    in_maps = make_in_maps(node_features, adj, W_embed, W1, Ws1, W2, Ws2)
    res = run_bass_kernel_spmd(nc, in_maps, core_ids=list(range(NCORES)), **spmd_kwargs)
    h = np.concatenate([r["h_out"] for r in res.results], axis=0)
    enc = np.concatenate([r["enc_out"] for r in res.results], axis=0)[:, : N - 2, :]
    return (h, enc, np.asarray(adj, dtype=np.float32)), res


def kernel(node_features, adj, W_embed, W1, Ws1, W2, Ws2):
    out, _ = run(node_features, adj, W_embed, W1, Ws1, W2, Ws2)
    return out


# revision 38
# speedup vs baseline: 1.3735x; 1.0739x over previous
"""Trainium2 Bass kernel for nn_Encoder_68977174774136 (heterogeneous GCN encoder).

Math (per batch b):
    X = node_features @ W_embed                       # [N, H]
    adj_n[c] = adj[c] / max(rowsum(adj[c]), 1)        # [N, N] per edge type
    4 layers (2 stacks x 2):
        xw[c] = X @ (W[l,c]/6)                        # 1/6 = (1-alpha)/C
        X = relu(X @ (0.5*Ws[l]) + sum_c adj_n[c] @ xw[c])
    h = [mean, max, std(ddof=1)] over nodes           # [3H]
    return h, X[:, :-2], adj

Sharding: data-parallel over batch, 16 graphs per NeuronCore x 8 cores.

Device strategy (per core):
  - Everything is feature-major on chip: X_t = X^T is [H=128, N] so H sits
    exactly on the 128 partitions; matmuls contract over partitions.
  - The aggregation adj_n @ xw needs adjacency with the contraction index j on
    partitions, i.e. transposed.  The host stages adj as adj^T (a pure layout
    transform, like the node-feature transpose); ALL math on it (degree,
    normalization, message passing) runs on device:
      deg[i]  = ones^T-matmul over the j-partitioned tiles (fp8, exact)
      inv     = 1/max(deg, 1)                          (DVE)
      inv_rep = ones x inv rank-1 matmul               (PE broadcast)
      adj_nt  = adjT * inv_rep -> bf16                 (DVE, written in place
                into SBUF-resident tiles)
    Normalized transposed adjacency lives in SBUF (bf16) in an 8-batch
    rotating window, so the 4 GCN layers never re-touch HBM for adjacency.
  - All heavy matmuls are bf16 (PE runs plain fp32 at 1/4 rate), accumulated
    in f32 PSUM.  alpha/C constants are folded into the weights host-side.
  - Batches flow through a software-pipelined group wavefront (groups of 4,
    layer-major inside a group) with preprocessing for group g+1 interleaved
    between group g's layer units, so every engine's in-order stream mixes
    the two pipeline stages.
"""

import sys

sys.path.insert(0, "/opt/trn_rl_repo")

from contextlib import ExitStack

import ml_dtypes
import numpy as np

import concourse.bacc as bacc
import concourse.mybir as mybir
import concourse.tile as tile
from concourse import masks
from concourse.bass_utils import run_bass_kernel_spmd

BB = 16  # batches per core
NCORES = 8
N = 402  # nodes
C = 3  # edge categories
H = 128  # hidden size == SBUF partitions
FIN = 6  # raw feature dim
NL = 4  # total GCN layers (2 stacks x 2)

F32 = mybir.dt.float32
FP8 = mybir.dt.float8e4
F16 = mybir.dt.float16
BF16 = mybir.dt.bfloat16
AF = mybir.ActivationFunctionType
AX = mybir.AxisListType
OP = mybir.AluOpType

# j (and i) tiled into chunks of <=128 for the 128-wide contraction
JCHUNKS = [(0, 128), (128, 128), (256, 128), (384, N - 384)]


def build_nc(rounds=1):
    nc = bacc.Bacc("TRN2", target_bir_lowering=False, debug=False)

    adjt = nc.dram_tensor("adjt", [BB, C, N, N], FP8, kind="ExternalInput")
    nf = nc.dram_tensor("nf", [BB, FIN, N], BF16, kind="ExternalInput")
    wmsg = nc.dram_tensor("wmsg", [NL, H, C * H], BF16, kind="ExternalInput")
    wself = nc.dram_tensor("wself", [NL, H, H], BF16, kind="ExternalInput")
    wemb = nc.dram_tensor("wemb", [FIN, H], BF16, kind="ExternalInput")
    enc_out = nc.dram_tensor("enc_out", [BB, N, H], F32, kind="ExternalOutput")
    h_out = nc.dram_tensor("h_out", [BB, 3 * H], F32, kind="ExternalOutput")

    with tile.TileContext(nc) as tc, ExitStack() as ctx:
        persist = ctx.enter_context(tc.tile_pool(name="persist", bufs=1))
        work = ctx.enter_context(tc.tile_pool(name="work", bufs=2))
        psp = ctx.enter_context(tc.tile_pool(name="psp", bufs=1, space="PSUM"))

        # ---- persistent tiles -----------------------------------------------
        X_t = [persist.tile([H, N], BF16, name=f"xt_{b}", tag=f"xt_{b}") for b in range(BB)]
        wmsg_sb = [
            persist.tile([H, C * H], BF16, name=f"wmsg_{l}", tag=f"wmsg_{l}") for l in range(NL)
        ]
        wself_sb = [
            persist.tile([H, H], BF16, name=f"wself_{l}", tag=f"wself_{l}") for l in range(NL)
        ]
        wemb_sb = persist.tile([FIN, H], BF16, name="wemb_sb", tag="wemb_sb")
        ident = persist.tile([H, H], F32, name="ident", tag="ident")
        ones_col = persist.tile([H, 1], FP8, name="ones_col", tag="ones_col")
        ones_row = persist.tile([1, H], F16, name="ones_row", tag="ones_row")
        s_sum = persist.tile([H, BB], F32, name="s_sum", tag="s_sum")
        s_max = persist.tile([H, BB], F32, name="s_max", tag="s_max")
        s_sq = persist.tile([H, BB], F32, name="s_sq", tag="s_sq")
        stats_cat = persist.tile([H, 3 * BB], F32, name="stats_cat", tag="stats_cat")
        tmp_a = persist.tile([H, BB], F32, name="tmp_a", tag="tmp_a")
        tmp_b = persist.tile([H, BB], F32, name="tmp_b", tag="tmp_b")

        # adj_nt[(b, c)][p, jc, i] = adj_n[b, c, i, jc*128+p] (bf16).  Only ~2
        # batch-groups are live at once, so slots rotate on b mod 8; Tile's
        # WAR tracking delays the normalize write of b+8 until the msg
        # matmuls of b finished reading the slot.
        adj_nt = {}

        # ---- constants and weights ------------------------------------------
        masks.make_identity(nc, ident[:, :])
        nc.gpsimd.memset(ones_col[:, :], 1.0)
        nc.gpsimd.memset(ones_row[:, :], 1.0)
        nc.sync.dma_start(wemb_sb[:, :], wemb[:, :])
        for l in range(NL):
            nc.sync.dma_start(wmsg_sb[l][:, :], wmsg[l])
            nc.sync.dma_start(wself_sb[l][:, :], wself[l])

        def act_copy(out, in_):
            nc.scalar.activation(out, in_, AF.Copy)

        # ---- per-(b, c) preprocessing unit ----------------------------------
        pre_count = [0]

        def embed_unit(b):
            # embed: X_t[b] = (nf[b] @ W_embed)^T via f-contraction
            nfs = work.tile([FIN, N], BF16, tag="nfsb", bufs=2)
            nc.sync.dma_start(nfs[:, :], nf[b])
            emb_ps = psp.tile([H, N], F32, tag="mm", bufs=3)
            nc.tensor.matmul(
                emb_ps[:, :], wemb_sb[:, :], nfs[:, :], start=True, stop=True
            )
            act_copy(X_t[b][:, :], emb_ps[:, :])

        def pre_unit_a(b, c):
            # adjT[b, c] rows are j; chunks land on partitions
            stage = work.tile([H, 4, N], FP8, tag="stage", bufs=8)
            if pre_count[0] < 8:
                # first touch of each pool slot: the chunk-3 tail partitions
                # are never DMA'd; init once so the deg matmul AP is clean
                nc.gpsimd.memset(stage[:, 3, :], 0.0)
            pre_count[0] += 1
            nc.sync.dma_start(
                stage[:, 0:3, :],
                adjt[b, c, 0:384, :].rearrange("(jc p) i -> p jc i", p=128),
            )
            nc.sync.dma_start(stage[0 : N - 384, 3, :], adjt[b, c, 384:N, :])

            # deg[i] = sum_j adjT[j, i] via ones-matmul (fp8 operands are
            # exact 0/1; f32 PSUM accumulation makes the counts exact)
            deg_ps = psp.tile([1, N], F32, tag="bc", bufs=2)
            for jc, (j0, jsz) in enumerate(JCHUNKS):
                nc.tensor.matmul(
                    deg_ps[0:1, :],
                    ones_col[0:jsz, :],
                    stage[0:jsz, jc, :],
                    start=(jc == 0),
                    stop=(jc == 3),
                )
            inv = work.tile([1, N], F32, tag="inv", bufs=4)
            nc.vector.tensor_scalar_max(inv[0:1, :], deg_ps[0:1, :], 1.0)
            nc.vector.reciprocal(inv[0:1, :], inv[0:1, :])
            inv16 = work.tile([1, N], F16, tag="inv16", bufs=4)
            nc.vector.tensor_copy(inv16[0:1, :], inv[0:1, :])
            return stage, inv16

        def pre_unit_b(b, c, stage, inv16):
            # replicate inv across partitions with a rank-1 f16 matmul (a full
            # wavefront step after the recip, so PE never waits on DVE here)
            bc_ps = psp.tile([H, N], F32, tag="bc", bufs=2)
            nc.tensor.matmul(
                bc_ps[:, :],
                ones_row[0:1, :],
                inv16[0:1, :],
                start=True,
                stop=True,
            )
            # normalize + cast straight into the SBUF-resident transposed tile
            ant = work.tile(
                [H, 4, N], BF16, name=f"adjnt_{b}_{c}", tag=f"adjnt_{b % 8}_{c}", bufs=1
            )
            adj_nt[(b, c)] = ant
            for jc, (j0, jsz) in enumerate(JCHUNKS):
                nc.vector.tensor_tensor(
                    ant[0:jsz, jc, 0:N],
                    stage[0:jsz, jc, :],
                    bc_ps[0:jsz, :],
                    op=OP.mult,
                )

        # ---- per-(b, l) layer unit ------------------------------------------
        def layer_unit(b, l):
            xws = []
            for jc, (j0, jsz) in enumerate(JCHUNKS):
                xw_ps = psp.tile([H, C * H], F32, tag="xw", bufs=3)
                nc.tensor.matmul(
                    xw_ps[0:jsz, :],
                    X_t[b][:, j0 : j0 + jsz],
                    wmsg_sb[l][:, :],
                    start=True,
                    stop=True,
                )
                xw_sb = work.tile([H, C * H], BF16, tag="xwsb", bufs=8)
                act_copy(xw_sb[0:jsz, :], xw_ps[0:jsz, :])
                xws.append(xw_sb)

            mm_ps = psp.tile([H, N], F32, tag="mm", bufs=3)
            nc.tensor.matmul(
                mm_ps[:, :], wself_sb[l][:, :], X_t[b][:, :], start=True, stop=False
            )
            cnt = 0
            # jc-outer so the first message matmuls only need xw chunk 0's
            # PSUM->SBUF copy; chunk k's copy has k*3*402 cycles of cover
            for jc, (j0, jsz) in enumerate(JCHUNKS):
                for c in range(C):
                    cnt += 1
                    nc.tensor.matmul(
                        mm_ps[:, :],
                        xws[jc][0:jsz, c * H : (c + 1) * H],
                        adj_nt[(b, c)][0:jsz, jc, 0:N],
                        start=False,
                        stop=(cnt == C * 4),
                    )

            if l < NL - 1:
                nc.scalar.activation(X_t[b][:, :], mm_ps[:, :], AF.Relu)
                return
            # final layer: f32 activations + readout stats + output
            xf = work.tile([H, N], F32, tag="xf32", bufs=2)
            nc.scalar.activation(
                xf[:, :], mm_ps[:, :], AF.Relu, accum_out=s_sum[:, b : b + 1]
            )
            nc.vector.reduce_max(s_max[:, b : b + 1], xf[:, :], axis=AX.X)
            junk = work.tile([H, N], BF16, tag="junk", bufs=2)
            nc.scalar.activation(
                junk[:, 0:N], xf[:, :], AF.Square, accum_out=s_sq[:, b : b + 1]
            )
            est = work.tile([H, 4, H], F32, tag="encst", bufs=2)
            for ic, (i0, isz) in enumerate(JCHUNKS):
                tp_ps = psp.tile([H, H], F32, tag="mm", bufs=3)
                nc.tensor.transpose(tp_ps[0:isz, :], xf[:, i0 : i0 + isz], ident[:, :])
                nc.vector.tensor_copy(est[0:isz, ic, :], tp_ps[0:isz, :])
            nc.sync.dma_start(
                enc_out[b, 0:384, :].rearrange("(ic p) g -> p ic g", p=128),
                est[:, 0:3, :],
            )
            nc.sync.dma_start(enc_out[b, 384:N, :], est[0 : N - 384, 3, :])

        # ---- software pipeline: group wavefront -----------------------------
        # Preprocessing is split: _a (loads + degree) and _b (broadcast +
        # normalize) are emitted one wavefront step apart so PE's in-order
        # stream never waits on the DVE reciprocal.
        GW = 4
        NSTEP = NL * GW
        for rnd in range(rounds):
            for b in range(BB):
                embed_unit(b)
            pre_list = [(b, c) for b in range(BB) for c in range(C)]
            pi = 0
            pending = {}
            while pi < GW * C:  # prologue: group 0
                st = pre_unit_a(*pre_list[pi])
                pre_unit_b(*pre_list[pi], *st)
                pi += 1
            for g in range(BB // GW):
                # schedule group g+1's 12 pre-units across this group's 16
                # layer steps: _a at step u*16//12, _b one step later
                sched = [[] for _ in range(NSTEP)]
                if pi < len(pre_list):
                    for u in range(GW * C):
                        sa = u * NSTEP // (GW * C)
                        sched[sa].append(("a", pre_list[pi + u]))
                        sched[min(sa + 1, NSTEP - 1)].append(("b", pre_list[pi + u]))
                    pi += GW * C
                step = 0
                for l in range(NL):
                    for b in range(g * GW, (g + 1) * GW):
                        layer_unit(b, l)
                        for kind, unit in sched[step]:
                            if kind == "a":
                                pending[unit] = pre_unit_a(*unit)
                            else:
                                pre_unit_b(*unit, *pending.pop(unit))
                        step += 1

        # ---- readout h = [mean | max | std] ---------------------------------
        nc.vector.tensor_scalar_mul(stats_cat[:, 0:BB], s_sum[:, :], 1.0 / N)
        nc.vector.tensor_copy(stats_cat[:, BB : 2 * BB], s_max[:, :])
        # var = ss/(N-1) - s^2/(N*(N-1)), clamped at 0
        nc.vector.tensor_mul(tmp_a[:, :], s_sum[:, :], s_sum[:, :])
        nc.vector.tensor_scalar_mul(tmp_b[:, :], s_sq[:, :], 1.0 / (N - 1))
        nc.vector.scalar_tensor_tensor(
            tmp_a[:, :], tmp_a[:, :], -1.0 / (N * (N - 1)), tmp_b[:, :], OP.mult, OP.add
        )
        nc.vector.tensor_scalar_max(tmp_a[:, :], tmp_a[:, :], 0.0)
        nc.scalar.activation(stats_cat[:, 2 * BB : 3 * BB], tmp_a[:, :], AF.Sqrt)

        tp_ps = psp.tile([H, H], F32, tag="mm", bufs=3)
        nc.tensor.transpose(tp_ps[0 : 3 * BB, :], stats_cat[:, :], ident[:, :])
        statsT = work.tile([H, H], F32, tag="statsT", bufs=1)
        nc.vector.tensor_copy(statsT[0 : 3 * BB, :], tp_ps[0 : 3 * BB, :])
        for s in range(3):
            nc.sync.dma_start(
                h_out[0:BB, s * H : (s + 1) * H], statsT[s * BB : (s + 1) * BB, 0:H]
            )

    nc.compile()
    return nc


_NC_CACHE = None


def get_nc():
    global _NC_CACHE
    if _NC_CACHE is None:
        _NC_CACHE = build_nc()
    return _NC_CACHE


def make_in_maps(node_features, adj, W_embed, W1, Ws1, W2, Ws2):
    node_features = np.asarray(node_features, dtype=np.float32)
    adj = np.asarray(adj, dtype=np.float32)
    W_embed = np.asarray(W_embed, dtype=np.float32)
    W1 = np.asarray(W1, dtype=np.float32)
    Ws1 = np.asarray(Ws1, dtype=np.float32)
    W2 = np.asarray(W2, dtype=np.float32)
    Ws2 = np.asarray(Ws2, dtype=np.float32)

    # layout staging only (no math): adjacency transposed so the contraction
    # index j is outermost, features transposed to feature-major
    adjT = np.ascontiguousarray(np.transpose(adj, (0, 1, 3, 2))).astype(
        ml_dtypes.float8_e4m3fn
    )
    nfT = np.ascontiguousarray(np.transpose(node_features, (0, 2, 1))).astype(
        ml_dtypes.bfloat16
    )
    # [NL, H, C*H]: (1-alpha)/C folded in; free dim is (c, g) c-major
    wm = np.concatenate([W1, W2], axis=0)  # [NL, C, H, H]
    wm = np.ascontiguousarray(
        (np.transpose(wm, (0, 2, 1, 3)) / 6.0).reshape(NL, H, C * H)
    ).astype(ml_dtypes.bfloat16)
    ws = (np.concatenate([Ws1, Ws2], axis=0) * 0.5).astype(ml_dtypes.bfloat16)
    we = W_embed.astype(ml_dtypes.bfloat16)

    in_maps = []
    for cid in range(NCORES):
        sl = slice(cid * BB, (cid + 1) * BB)
        in_maps.append(
            {
                "adjt": np.ascontiguousarray(adjT[sl]),
                "nf": np.ascontiguousarray(nfT[sl]),
                "wmsg": wm,
                "wself": ws,
                "wemb": we,
            }
        )
    return in_maps


def run(node_features, adj, W_embed, W1, Ws1, W2, Ws2, **spmd_kwargs):
    nc = get_nc()
    in_maps = make_in_maps(node_features, adj, W_embed, W1, Ws1, W2, Ws2)
    res = run_bass_kernel_spmd(nc, in_maps, core_ids=list(range(NCORES)), **spmd_kwargs)
    h = np.concatenate([r["h_out"] for r in res.results], axis=0)
    enc = np.concatenate([r["enc_out"] for r in res.results], axis=0)[:, : N - 2, :]
    return (h, enc, np.asarray(adj, dtype=np.float32)), res


def kernel(node_features, adj, W_embed, W1, Ws1, W2, Ws2):
    out, _ = run(node_features, adj, W_embed, W1, Ws1, W2, Ws2)
    return out


# revision 39
# speedup vs baseline: 1.5416x; 1.1224x over previous
"""Trainium2 Bass kernel for nn_Encoder_68977174774136 (heterogeneous GCN encoder).

Math (per batch b):
    X = node_features @ W_embed                       # [N, H]
    adj_n[c] = adj[c] / max(rowsum(adj[c]), 1)        # [N, N] per edge type
    4 layers (2 stacks x 2):
        xw[c] = X @ (W[l,c]/6)                        # 1/6 = (1-alpha)/C
        X = relu(X @ (0.5*Ws[l]) + sum_c adj_n[c] @ xw[c])
    h = [mean, max, std(ddof=1)] over nodes           # [3H]
    return h, X[:, :-2], adj

Sharding: data-parallel over batch, 16 graphs per NeuronCore x 8 cores.

Device strategy (per core):
  - Everything is feature-major on chip: X_t = X^T is [H=128, N] so H sits
    exactly on the 128 partitions; matmuls contract over partitions.
  - The aggregation adj_n @ xw needs adjacency with the contraction index j on
    partitions, i.e. transposed.  The host stages adj as adj^T (a pure layout
    transform, like the node-feature transpose); ALL math on it (degree,
    normalization, message passing) runs on device:
      deg[i]  = ones^T-matmul over the j-partitioned tiles (fp8, exact)
      inv     = 1/max(deg, 1)                          (DVE)
      inv_rep = ones x inv rank-1 matmul               (PE broadcast)
      adj_nt  = adjT * inv_rep -> bf16                 (DVE, written in place
                into SBUF-resident tiles)
    Normalized transposed adjacency lives in SBUF (bf16) in an 8-batch
    rotating window, so the 4 GCN layers never re-touch HBM for adjacency.
  - All heavy matmuls are bf16 (PE runs plain fp32 at 1/4 rate), accumulated
    in f32 PSUM.  alpha/C constants are folded into the weights host-side.
  - Batches flow through a software-pipelined group wavefront (groups of 4,
    layer-major inside a group) with preprocessing for group g+1 interleaved
    between group g's layer units, so every engine's in-order stream mixes
    the two pipeline stages.
"""

import sys

sys.path.insert(0, "/opt/trn_rl_repo")

from contextlib import ExitStack

import ml_dtypes
import numpy as np

import concourse.bacc as bacc
import concourse.mybir as mybir
import concourse.tile as tile
from concourse import masks
from concourse.bass_utils import run_bass_kernel_spmd

BB = 16  # batches per core
NCORES = 8
N = 402  # nodes
C = 3  # edge categories
H = 128  # hidden size == SBUF partitions
FIN = 6  # raw feature dim
NL = 4  # total GCN layers (2 stacks x 2)

F32 = mybir.dt.float32
FP8 = mybir.dt.float8e4
F16 = mybir.dt.float16
BF16 = mybir.dt.bfloat16
AF = mybir.ActivationFunctionType
AX = mybir.AxisListType
OP = mybir.AluOpType

# j (and i) tiled into chunks of <=128 for the 128-wide contraction
JCHUNKS = [(0, 128), (128, 128), (256, 128), (384, N - 384)]


def build_nc(rounds=1):
    nc = bacc.Bacc("TRN2", target_bir_lowering=False, debug=False)

    adjt = nc.dram_tensor("adjt", [BB, C, N, N], FP8, kind="ExternalInput")
    nf = nc.dram_tensor("nf", [BB, FIN, N], BF16, kind="ExternalInput")
    wmsg = nc.dram_tensor("wmsg", [NL, H, C * H], BF16, kind="ExternalInput")
    wself = nc.dram_tensor("wself", [NL, H, H], BF16, kind="ExternalInput")
    wemb = nc.dram_tensor("wemb", [FIN, H], BF16, kind="ExternalInput")
    enc_out = nc.dram_tensor("enc_out", [BB, N, H], F32, kind="ExternalOutput")
    h_out = nc.dram_tensor("h_out", [BB, 3 * H], F32, kind="ExternalOutput")

    with tile.TileContext(nc) as tc, ExitStack() as ctx:
        persist = ctx.enter_context(tc.tile_pool(name="persist", bufs=1))
        work = ctx.enter_context(tc.tile_pool(name="work", bufs=2))
        psp = ctx.enter_context(tc.tile_pool(name="psp", bufs=1, space="PSUM"))

        # ---- persistent tiles -----------------------------------------------
        X_t = [persist.tile([H, N], BF16, name=f"xt_{b}", tag=f"xt_{b}") for b in range(BB)]
        wmsg_sb = [
            persist.tile([H, C * H], BF16, name=f"wmsg_{l}", tag=f"wmsg_{l}") for l in range(NL)
        ]
        wself_sb = [
            persist.tile([H, H], BF16, name=f"wself_{l}", tag=f"wself_{l}") for l in range(NL)
        ]
        wemb_sb = persist.tile([FIN, H], BF16, name="wemb_sb", tag="wemb_sb")
        ident = persist.tile([H, H], F32, name="ident", tag="ident")
        ones_col = persist.tile([H, 1], FP8, name="ones_col", tag="ones_col")
        s_sum = persist.tile([H, BB], F32, name="s_sum", tag="s_sum")
        s_max = persist.tile([H, BB], F32, name="s_max", tag="s_max")
        s_sq = persist.tile([H, BB], F32, name="s_sq", tag="s_sq")
        stats_cat = persist.tile([H, 3 * BB], F32, name="stats_cat", tag="stats_cat")
        tmp_a = persist.tile([H, BB], F32, name="tmp_a", tag="tmp_a")
        tmp_b = persist.tile([H, BB], F32, name="tmp_b", tag="tmp_b")

        # adj_nt[(b, c)][p, jc, i] = adj_n[b, c, i, jc*128+p] (bf16).  Only ~2
        # batch-groups are live at once, so slots rotate on b mod 8; Tile's
        # WAR tracking delays the normalize write of b+8 until the msg
        # matmuls of b finished reading the slot.
        adj_nt = {}

        # ---- constants and weights ------------------------------------------
        masks.make_identity(nc, ident[:, :])
        nc.gpsimd.memset(ones_col[:, :], 1.0)
        nc.sync.dma_start(wemb_sb[:, :], wemb[:, :])
        for l in range(NL):
            nc.sync.dma_start(wmsg_sb[l][:, :], wmsg[l])
            nc.sync.dma_start(wself_sb[l][:, :], wself[l])

        def act_copy(out, in_):
            nc.scalar.activation(out, in_, AF.Copy)

        # ---- per-(b, c) preprocessing unit ----------------------------------
        pre_count = [0]

        def embed_prologue():
            # burst all nf loads first, then the 16 embed matmuls, so PE only
            # waits on the first DMA
            nfs_all = []
            for b in range(BB):
                nfs = work.tile([FIN, N], BF16, tag=f"nfsb_{b}", bufs=1, name=f"nfs_{b}")
                nc.sync.dma_start(nfs[:, :], nf[b])
                nfs_all.append(nfs)
            for b in range(BB):
                emb_ps = psp.tile([H, N], F32, tag="mm", bufs=3)
                nc.tensor.matmul(
                    emb_ps[:, :], wemb_sb[:, :], nfs_all[b][:, :], start=True, stop=True
                )
                act_copy(X_t[b][:, :], emb_ps[:, :])

        def pre_unit_a(b, c):
            # adjT[b, c] rows are j; chunks land on partitions
            stage = work.tile([H, 4, N], FP8, tag="stage", bufs=8)
            if pre_count[0] < 8:
                # first touch of each pool slot: the chunk-3 tail partitions
                # are never DMA'd; init once so the deg matmul AP is clean
                nc.gpsimd.memset(stage[:, 3, :], 0.0)
            pre_count[0] += 1
            nc.sync.dma_start(
                stage[:, 0:3, :],
                adjt[b, c, 0:384, :].rearrange("(jc p) i -> p jc i", p=128),
            )
            nc.sync.dma_start(stage[0 : N - 384, 3, :], adjt[b, c, 384:N, :])

            # deg[i] = sum_j adjT[j, i] via ones-matmul (fp8 operands are
            # exact 0/1; f32 PSUM accumulation makes the counts exact)
            deg_ps = psp.tile([1, N], F32, tag="bc", bufs=2)
            for jc, (j0, jsz) in enumerate(JCHUNKS):
                nc.tensor.matmul(
                    deg_ps[0:1, :],
                    ones_col[0:jsz, :],
                    stage[0:jsz, jc, :],
                    start=(jc == 0),
                    stop=(jc == 3),
                )
            inv = work.tile([1, N], F32, tag="inv", bufs=4)
            nc.vector.tensor_scalar_max(inv[0:1, :], deg_ps[0:1, :], 1.0)
            nc.vector.reciprocal(inv[0:1, :], inv[0:1, :])
            return stage, inv

        def pre_unit_b(b, c, stage, inv):
            # replicate inv across partitions on the otherwise-idle GpSimd
            # engine (a wavefront step after the recip), freeing PE cycles
            bc_ps = work.tile([H, N], F32, tag="bcsb", bufs=2)
            nc.gpsimd.partition_broadcast(bc_ps[:, :], inv[0:1, :])
            # normalize + cast straight into the SBUF-resident transposed tile
            ant = work.tile(
                [H, 4, N], BF16, name=f"adjnt_{b}_{c}", tag=f"adjnt_{b % 8}_{c}", bufs=1
            )
            adj_nt[(b, c)] = ant
            for jc, (j0, jsz) in enumerate(JCHUNKS):
                nc.vector.tensor_tensor(
                    ant[0:jsz, jc, 0:N],
                    stage[0:jsz, jc, :],
                    bc_ps[0:jsz, :],
                    op=OP.mult,
                )

        # ---- per-(b, l) layer unit ------------------------------------------
        def layer_unit(b, l):
            xws = []
            for jc, (j0, jsz) in enumerate(JCHUNKS):
                xw_ps = psp.tile([H, C * H], F32, tag="xw", bufs=3)
                nc.tensor.matmul(
                    xw_ps[0:jsz, :],
                    X_t[b][:, j0 : j0 + jsz],
                    wmsg_sb[l][:, :],
                    start=True,
                    stop=True,
                )
                xw_sb = work.tile([H, C * H], BF16, tag="xwsb", bufs=8)
                act_copy(xw_sb[0:jsz, :], xw_ps[0:jsz, :])
                xws.append(xw_sb)

            mm_ps = psp.tile([H, N], F32, tag="mm", bufs=3)
            nc.tensor.matmul(
                mm_ps[:, :], wself_sb[l][:, :], X_t[b][:, :], start=True, stop=False
            )
            cnt = 0
            # jc-outer so the first message matmuls only need xw chunk 0's
            # PSUM->SBUF copy; chunk k's copy has k*3*402 cycles of cover
            for jc, (j0, jsz) in enumerate(JCHUNKS):
                for c in range(C):
                    cnt += 1
                    nc.tensor.matmul(
                        mm_ps[:, :],
                        xws[jc][0:jsz, c * H : (c + 1) * H],
                        adj_nt[(b, c)][0:jsz, jc, 0:N],
                        start=False,
                        stop=(cnt == C * 4),
                    )

            if l < NL - 1:
                nc.scalar.activation(X_t[b][:, :], mm_ps[:, :], AF.Relu)
                return
            # final layer: f32 activations + readout stats + output
            xf = work.tile([H, N], F32, tag="xf32", bufs=2)
            nc.scalar.activation(
                xf[:, :], mm_ps[:, :], AF.Relu, accum_out=s_sum[:, b : b + 1]
            )
            nc.vector.reduce_max(s_max[:, b : b + 1], xf[:, :], axis=AX.X)
            junk = work.tile([H, N], BF16, tag="junk", bufs=2)
            nc.scalar.activation(
                junk[:, 0:N], xf[:, :], AF.Square, accum_out=s_sq[:, b : b + 1]
            )
            est = work.tile([H, 4, H], F32, tag="encst", bufs=2)
            for ic, (i0, isz) in enumerate(JCHUNKS):
                tp_ps = psp.tile([H, H], F32, tag="mm", bufs=3)
                nc.tensor.transpose(tp_ps[0:isz, :], xf[:, i0 : i0 + isz], ident[:, :])
                nc.vector.tensor_copy(est[0:isz, ic, :], tp_ps[0:isz, :])
            nc.sync.dma_start(
                enc_out[b, 0:384, :].rearrange("(ic p) g -> p ic g", p=128),
                est[:, 0:3, :],
            )
            nc.sync.dma_start(enc_out[b, 384:N, :], est[0 : N - 384, 3, :])

        # ---- software pipeline: group wavefront -----------------------------
        # Preprocessing is split: _a (loads + degree) and _b (broadcast +
        # normalize) are emitted one wavefront step apart so PE's in-order
        # stream never waits on the DVE reciprocal.
        GW = 4
        NSTEP = NL * GW
        for rnd in range(rounds):
            embed_prologue()
            pre_list = [(b, c) for b in range(BB) for c in range(C)]
            pi = 0
            pending = {}
            while pi < GW * C:  # prologue: group 0
                st = pre_unit_a(*pre_list[pi])
                pre_unit_b(*pre_list[pi], *st)
                pi += 1
            for g in range(BB // GW):
                # schedule group g+1's 12 pre-units across this group's 16
                # layer steps: _a at step u*16//12, _b one step later
                sched = [[] for _ in range(NSTEP)]
                if pi < len(pre_list):
                    for u in range(GW * C):
                        sa = u * NSTEP // (GW * C)
                        sched[sa].append(("a", pre_list[pi + u]))
                        sched[min(sa + 1, NSTEP - 1)].append(("b", pre_list[pi + u]))
                    pi += GW * C
                step = 0
                for l in range(NL):
                    for b in range(g * GW, (g + 1) * GW):
                        layer_unit(b, l)
                        for kind, unit in sched[step]:
                            if kind == "a":
                                pending[unit] = pre_unit_a(*unit)
                            else:
                                pre_unit_b(*unit, *pending.pop(unit))
                        step += 1

        # ---- readout h = [mean | max | std] ---------------------------------
        nc.vector.tensor_scalar_mul(stats_cat[:, 0:BB], s_sum[:, :], 1.0 / N)
        nc.vector.tensor_copy(stats_cat[:, BB : 2 * BB], s_max[:, :])
        # var = ss/(N-1) - s^2/(N*(N-1)), clamped at 0
        nc.vector.tensor_mul(tmp_a[:, :], s_sum[:, :], s_sum[:, :])
        nc.vector.tensor_scalar_mul(tmp_b[:, :], s_sq[:, :], 1.0 / (N - 1))
        nc.vector.scalar_tensor_tensor(
            tmp_a[:, :], tmp_a[:, :], -1.0 / (N * (N - 1)), tmp_b[:, :], OP.mult, OP.add
        )
        nc.vector.tensor_scalar_max(tmp_a[:, :], tmp_a[:, :], 0.0)
        nc.scalar.activation(stats_cat[:, 2 * BB : 3 * BB], tmp_a[:, :], AF.Sqrt)

        tp_ps = psp.tile([H, H], F32, tag="mm", bufs=3)
        nc.tensor.transpose(tp_ps[0 : 3 * BB, :], stats_cat[:, :], ident[:, :])
        statsT = work.tile([H, H], F32, tag="statsT", bufs=1)
        nc.vector.tensor_copy(statsT[0 : 3 * BB, :], tp_ps[0 : 3 * BB, :])
        for s in range(3):
            nc.sync.dma_start(
                h_out[0:BB, s * H : (s + 1) * H], statsT[s * BB : (s + 1) * BB, 0:H]
            )

    nc.compile()
    return nc


_NC_CACHE = None


def get_nc():
    global _NC_CACHE
    if _NC_CACHE is None:
        _NC_CACHE = build_nc()
    return _NC_CACHE


def make_in_maps(node_features, adj, W_embed, W1, Ws1, W2, Ws2):
    node_features = np.asarray(node_features, dtype=np.float32)
    adj = np.asarray(adj, dtype=np.float32)
    W_embed = np.asarray(W_embed, dtype=np.float32)
    W1 = np.asarray(W1, dtype=np.float32)
    Ws1 = np.asarray(Ws1, dtype=np.float32)
    W2 = np.asarray(W2, dtype=np.float32)
    Ws2 = np.asarray(Ws2, dtype=np.float32)

    # layout staging only (no math): adjacency transposed so the contraction
    # index j is outermost, features transposed to feature-major
    adjT = np.ascontiguousarray(np.transpose(adj, (0, 1, 3, 2))).astype(
        ml_dtypes.float8_e4m3fn
    )
    nfT = np.ascontiguousarray(np.transpose(node_features, (0, 2, 1))).astype(
        ml_dtypes.bfloat16
    )
    # [NL, H, C*H]: (1-alpha)/C folded in; free dim is (c, g) c-major
    wm = np.concatenate([W1, W2], axis=0)  # [NL, C, H, H]
    wm = np.ascontiguousarray(
        (np.transpose(wm, (0, 2, 1, 3)) / 6.0).reshape(NL, H, C * H)
    ).astype(ml_dtypes.bfloat16)
    ws = (np.concatenate([Ws1, Ws2], axis=0) * 0.5).astype(ml_dtypes.bfloat16)
    we = W_embed.astype(ml_dtypes.bfloat16)

    in_maps = []
    for cid in range(NCORES):
        sl = slice(cid * BB, (cid + 1) * BB)
        in_maps.append(
            {
                "adjt": np.ascontiguousarray(adjT[sl]),
                "nf": np.ascontiguousarray(nfT[sl]),
                "wmsg": wm,
                "wself": ws,
                "wemb": we,
            }
        )
    return in_maps


def run(node_features, adj, W_embed, W1, Ws1, W2, Ws2, **spmd_kwargs):
    nc = get_nc()
    in_maps = make_in_maps(node_features, adj, W_embed, W1, Ws1, W2, Ws2)
    res = run_bass_kernel_spmd(nc, in_maps, core_ids=list(range(NCORES)), **spmd_kwargs)
    h = np.concatenate([r["h_out"] for r in res.results], axis=0)
    enc = np.concatenate([r["enc_out"] for r in res.results], axis=0)[:, : N - 2, :]
    return (h, enc, np.asarray(adj, dtype=np.float32)), res


def kernel(node_features, adj, W_embed, W1, Ws1, W2, Ws2):
    out, _ = run(node_features, adj, W_embed, W1, Ws1, W2, Ws2)
    return out
